# revision 36
# baseline (speedup 1.0000x reference)
"""Multi-head causal attention (bs=4, L=2048, d_model=512, 8 heads x 64) on 8
Trainium2 NeuronCores.

Sharding: core c = (batch b = c//2, head-group hg = c%2); each core computes 4
heads of one batch over the full sequence.

v3: f16 everywhere except the q/k score operands, which are stored fp8e4 so
the score matmuls can run in DoubleRow perf mode (2 rows/PE-cycle) with the
64-deep head contraction padded by a shared zero plane (softmax weight noise
transfers ~1:1 to the output, so fp8 is only affordable on scores, where it
enters through s*~0.2-magnitude logits). z matmuls carry a ones-column (M=65)
so denominators fall out of PSUM; diagonal blocks use column-trimmed score/z
matmuls, constant-triangle masks and small dead-column memsets. 1/sqrt(dk) is
folded into w_q host-side. Output is f16 (summed/transposed on host).
"""

import numpy as np
import ml_dtypes

import concourse.bacc as bacc
import concourse.mybir as mybir
import concourse.tile as tile
from concourse.bass_utils import run_bass_kernel_spmd

F32 = mybir.dt.float32
F16 = mybir.dt.float16
F8 = mybir.dt.float8e4
AF = mybir.ActivationFunctionType
DR = mybir.MatmulPerfMode.DoubleRow
ALU = mybir.AluOpType

L = 2048
D = 512
HD = 256
DK = 64
NH = 4
P = 128
IB = 512
NIB = L // IB          # 4 query blocks
NKT = D // P           # 4 model-dim tiles
ZW = DK + 1            # z matmul M (64 v dims + ones row -> denominator)

TRI_GPS_EVERY = 0      # every k-th triangle mask runs on gpsimd


def _build():
    nc = bacc.Bacc("TRN2", target_bir_lowering=False, debug=False,
                   enable_asserts=False)

    xT = nc.dram_tensor("xT", [D, L], F16, kind="ExternalInput")
    wq = nc.dram_tensor("wq", [D, HD], F16, kind="ExternalInput")
    wk = nc.dram_tensor("wk", [D, HD], F16, kind="ExternalInput")
    wv = nc.dram_tensor("wv", [D, HD], F16, kind="ExternalInput")
    wo = nc.dram_tensor("wo", [HD, D], F16, kind="ExternalInput")
    bq = nc.dram_tensor("bq", [HD], F32, kind="ExternalInput")
    bk = nc.dram_tensor("bk", [HD], F32, kind="ExternalInput")
    outT = nc.dram_tensor("outT", [D, L], F16, kind="ExternalOutput")
    scr2 = nc.dram_tensor("scr2", [NIB * NH, IB], F32, kind="Internal")

    counters = {"tri": 0}

    with tile.TileContext(nc) as tc:
        with (
            tc.tile_pool(name="w", bufs=1) as pw,
            tc.tile_pool(name="at", bufs=6) as pats,
            tc.tile_pool(name="zc", bufs=2) as pzc,
            tc.tile_pool(name="nm", bufs=2) as pnm,
            tc.tile_pool(name="bc", bufs=2) as pbct,
            tc.tile_pool(name="o", bufs=2) as posb,
            tc.tile_pool(name="ps", bufs=2, space="PSUM") as pps,
            tc.tile_pool(name="pz", bufs=2, space="PSUM") as ppz,
            tc.tile_pool(name="pp", bufs=2, space="PSUM") as ppp,
        ):
            # ---- loads ----
            wq_sb = pw.tile([P, NKT, HD], F16, tag="wq")
            wk_sb = pw.tile([P, NKT, HD], F16, tag="wk")
            wv_sb = pw.tile([P, NKT, HD], F16, tag="wv")
            wo_sb = pw.tile([P, HD // P, D], F16, tag="wo")
            bq_sb = pw.tile([P, HD // P], F32, tag="bq")
            bk_sb = pw.tile([P, HD // P], F32, tag="bk")
            nc.sync.dma_start(bq_sb[:], bq.ap().rearrange("(t p) -> p t", p=P))
            nc.sync.dma_start(bk_sb[:], bk.ap().rearrange("(t p) -> p t", p=P))
            # interleave per-k-tile weight/x chunks so qkproj kt=t can start
            # as soon as its slice lands instead of after whole-tensor loads
            xt = pw.tile([P, NKT, L], F16, tag="xt")
            for t in range(NKT):
                nc.sync.dma_start(wq_sb[:, t, :], wq.ap()[t * P:(t + 1) * P, :])
                nc.sync.dma_start(wk_sb[:, t, :], wk.ap()[t * P:(t + 1) * P, :])
                nc.sync.dma_start(xt[:, t, 0:IB],
                                  xT.ap()[t * P:(t + 1) * P, 0:IB])
            for ibx in range(1, NIB):
                for t in range(NKT):
                    nc.sync.dma_start(
                        xt[:, t, ibx * IB:(ibx + 1) * IB],
                        xT.ap()[t * P:(t + 1) * P, ibx * IB:(ibx + 1) * IB])
            for t in range(NKT):
                nc.sync.dma_start(wv_sb[:, t, :], wv.ap()[t * P:(t + 1) * P, :])
            nc.sync.dma_start(wo_sb[:], wo.ap().rearrange("(t p) n -> p t n", p=P))

            # ---- consts: triangle mask, fp8 q/k mega tile with zero plane ----
            # mega planes: 0,1 = q head-pairs (01, 23), 2,3 = k head-pairs,
            # 4 = zeros (the shared second DoubleRow k-tile). Head h lives on
            # partitions (h%2)*64 .. +64 of plane h//2 (q) / 2+h//2 (k).
            mega = pw.tile([P, 5, L], F8, tag="mega")
            nc.gpsimd.memset(mega[:, 4, :], 0.0)
            tri16 = pw.tile([P, P], F16, tag="tri16")
            nc.gpsimd.memset(tri16[:], 1.0)
            # keep element iff col - row >= 0
            nc.gpsimd.affine_select(tri16[:], tri16[:], pattern=[[1, P]],
                                    compare_op=ALU.is_ge, fill=0.0,
                                    base=0, channel_multiplier=-1)

            def q_ap(h, cols):
                b0 = (h % 2) * DK
                pl = h // 2
                return mega[b0:b0 + DK, pl:5:(4 - pl), cols]

            def k_ap(h, cols):
                b0 = (h % 2) * DK
                pl = 2 + h // 2
                return mega[b0:b0 + DK, pl:5:(4 - pl), cols]

            def qkproj(hp, is_k, ib):
                plane = (2 + hp) if is_k else hp
                w_sb = wk_sb if is_k else wq_sb
                b_sb = bk_sb if is_k else bq_sb
                pq = ppp.tile([P, IB], F32, tag="pp", name="pp")
                for t in range(NKT):
                    nc.tensor.matmul(
                        pq[:],
                        lhsT=w_sb[:, t, hp * P:(hp + 1) * P],
                        rhs=xt[:, t, ib * IB:(ib + 1) * IB],
                        start=(t == 0), stop=(t == NKT - 1))
                nc.vector.tensor_scalar_add(
                    mega[:, plane, ib * IB:(ib + 1) * IB], pq[:],
                    b_sb[:, hp:hp + 1])

            vts = {}

            def vproj(jp):
                vt = pw.tile([P, 2, NH, ZW], F16, tag=f"v{jp}", name=f"v{jp}")
                vts[jp] = vt
                for s in range(2):
                    jt = 2 * jp + s
                    pv = ppp.tile([P, HD], F32, tag="pp", name="pp")
                    for t in range(NKT):
                        nc.tensor.matmul(
                            pv[:],
                            lhsT=xt[:, t, jt * P:(jt + 1) * P],
                            rhs=wv_sb[:, t, :],
                            start=(t == 0), stop=(t == NKT - 1))
                    nc.vector.tensor_copy(
                        vt[:, s, :, 0:DK],
                        pv[:].rearrange("p (h e) -> p h e", h=NH))
                nc.gpsimd.memset(vt[:, :, :, DK:ZW], 1.0)

            zcs = {}

            def attn(ib, h):
                nj = 4 * (ib + 1)
                pzt = ppz.tile([ZW, IB], F32, tag="pz", name="pz")
                pending = []

                def emit_z(jt, att, t, c0):
                    nc.tensor.matmul(
                        pzt[:, c0:IB],
                        lhsT=vts[jt // 2][:, jt % 2, h, :],
                        rhs=att[:, t, c0:IB],
                        start=(jt == 0), stop=(jt == nj - 1),
                        skip_group_check=True)

                for g in range(nj // 2):
                    pst = pps.tile([P, 2, IB], F32, tag="ps", name="ps")
                    att = pats.tile([P, 2, IB], F16, tag="at", name="at")
                    for t in range(2):
                        jt = 2 * g + t
                        d = jt - 4 * ib
                        c0 = 128 * d if d > 0 else 0
                        nc.tensor.matmul(
                            pst[:, t, c0:IB],
                            lhsT=k_ap(h, slice(jt * P, (jt + 1) * P)),
                            rhs=q_ap(h, slice(ib * IB + c0, (ib + 1) * IB)),
                            start=True, stop=True, perf_mode=DR)
                    # one exp per pair over the pair's live-col union;
                    # remaining garbage cols are never read by the
                    # column-trimmed z matmuls below
                    d0 = 2 * g - 4 * ib
                    cE = 128 * d0 if d0 > 0 else 0
                    nc.scalar.activation(att[:, :, cE:IB], pst[:, :, cE:IB],
                                         AF.Exp)
                    for t in range(2):
                        jt = 2 * g + t
                        d = jt - 4 * ib
                        c0 = 128 * d if d > 0 else 0
                        if d >= 0:
                            teng = (nc.gpsimd if TRI_GPS_EVERY and
                                    counters["tri"] % TRI_GPS_EVERY == 1
                                    else nc.vector)
                            counters["tri"] += 1
                            teng.tensor_mul(att[:, t, c0:c0 + P],
                                            att[:, t, c0:c0 + P], tri16[:])
                        if len(pending) >= 3:
                            emit_z(*pending.pop(0))
                        pending.append((jt, att, t, c0))
                for pg in pending:
                    emit_z(*pg)

                # ---- z+den out of PSUM; 1/den; bct via dram replicate ----
                idx = ib * NH + h
                dsb = pnm.tile([P, IB], F32, tag="dsb", name="dsb")
                nc.vector.tensor_copy(dsb[0:ZW, :], pzt[0:ZW, :])
                rec = pnm.tile([P, IB], F32, tag="rec", name="rec")
                rc2 = pnm.tile([P, IB], F32, tag="rc2", name="rc2")
                nc.sync.dma_start(rec[0:1, :], dsb[DK:ZW, :])
                nc.vector.reciprocal_approx_fast(rc2[0:1, :], rec[0:1, :])
                nc.sync.dma_start(scr2.ap()[idx:idx + 1, :], rc2[0:1, :])
                bct = pbct.tile([DK, IB], F32, tag="bct", name="bct")
                nc.gpsimd.dma_start(
                    bct[:], scr2.ap()[idx:idx + 1, :].broadcast_to([DK, IB]))

                zc = zcs[ib][h // 2]
                if h % 2 == 0:
                    nc.vector.tensor_mul(zc[0:DK, :], dsb[0:DK, :], bct[:])
                else:
                    zn = pnm.tile([DK, IB], F16, tag="zn", name="zn")
                    nc.vector.tensor_mul(zn[:], dsb[0:DK, :], bct[:])
                    nc.gpsimd.dma_start(zc[DK:P, :], zn[:])

            def new_zc(ib):
                zcs[ib] = [pzc.tile([P, IB], F16, tag=f"zc{i}", name=f"zc{i}")
                           for i in range(HD // P)]

            def outproj(ib):
                zc = zcs[ib]
                for mt in range(D // P):
                    po = ppp.tile([P, IB], F32, tag="pp", name="pp")
                    for k2 in range(HD // P):
                        nc.tensor.matmul(
                            po[:],
                            lhsT=wo_sb[:, k2, mt * P:(mt + 1) * P],
                            rhs=zc[k2][:],
                            start=(k2 == 0), stop=(k2 == HD // P - 1))
                    osb = posb.tile([P, IB], F16, tag="o", name="o")
                    nc.vector.tensor_copy(osb[:], po[:])
                    nc.gpsimd.dma_start(
                        outT.ap()[mt * P:(mt + 1) * P,
                                  ib * IB:(ib + 1) * IB], osb[:])

            # ---- emission schedule ----
            qkproj(0, True, 0)
            qkproj(0, False, 0)
            vproj(0)
            vproj(1)
            new_zc(0)
            attn(0, 1)
            qkproj(1, True, 0)
            qkproj(1, False, 0)
            attn(0, 0)
            attn(0, 3)
            for jp in range(2, 8):
                vproj(jp)
            attn(0, 2)
            for ib in range(1, NIB):
                for hp in range(2):
                    qkproj(hp, True, ib)
                    qkproj(hp, False, ib)
            outproj(0)
            for ib in range(1, NIB):
                new_zc(ib)
                for h in (1, 0, 3, 2):
                    attn(ib, h)
                outproj(ib)

    nc.compile()
    return nc


_NC = None


def _get_nc():
    global _NC
    if _NC is None:
        _NC = _build()
    return _NC


def _in_maps(x, w_q, b_q, w_k, b_k, w_v, b_v, w_o, b_o):
    scale = 1.0 / np.sqrt(DK)
    maps = []
    for b in range(4):
        xTb = np.ascontiguousarray(x[b].T).astype(np.float16)
        for hg in range(2):
            sl = slice(hg * HD, (hg + 1) * HD)
            maps.append({
                "xT": xTb,
                "wq": np.ascontiguousarray((w_q[sl] * scale).T).astype(np.float16),
                "wk": np.ascontiguousarray(w_k[sl].T).astype(np.float16),
                "wv": np.ascontiguousarray(w_v[sl].T).astype(np.float16),
                "wo": np.ascontiguousarray(w_o[:, sl].T).astype(np.float16),
                "bq": np.ascontiguousarray(b_q[sl] * scale).astype(np.float32),
                "bk": np.ascontiguousarray(b_k[sl]).astype(np.float32),
            })
    return maps


def _combine(results, w_o, b_v, b_o):
    corr = (b_o + w_o @ b_v).astype(np.float32)
    out = np.empty((4, L, D), dtype=np.float32)
    for b in range(4):
        acc = (results[2 * b]["outT"].astype(np.float32)
               + results[2 * b + 1]["outT"].astype(np.float32))
        out[b] = acc.T + corr
    return out


def kernel(x, w_q, b_q, w_k, b_k, w_v, b_v, w_o, b_o):
    nc = _get_nc()
    maps = _in_maps(x, w_q, b_q, w_k, b_k, w_v, b_v, w_o, b_o)
    res = run_bass_kernel_spmd(nc, maps, core_ids=list(range(8)))
    return _combine(res.results, w_o, b_v, b_o)


def bench(x, w_q, b_q, w_k, b_k, w_v, b_v, w_o, b_o):
    """Run with NTFF tracing; returns (output, exec_time_ns)."""
    nc = _get_nc()
    maps = _in_maps(x, w_q, b_q, w_k, b_k, w_v, b_v, w_o, b_o)
    res = run_bass_kernel_spmd(nc, maps, core_ids=list(range(8)), trace=True)
    return _combine(res.results, w_o, b_v, b_o), res.exec_time_ns


# revision 37
# speedup vs baseline: 1.0301x; 1.0301x over previous
"""Multi-head causal attention (bs=4, L=2048, d_model=512, 8 heads x 64) on 8
Trainium2 NeuronCores.

Sharding: core c = (batch b = c//2, head-group hg = c%2); each core computes 4
heads of one batch over the full sequence.

v3: f16 everywhere except the q/k score operands, which are stored fp8e4 so
the score matmuls can run in DoubleRow perf mode (2 rows/PE-cycle) with the
64-deep head contraction padded by a shared zero plane (softmax weight noise
transfers ~1:1 to the output, so fp8 is only affordable on scores, where it
enters through s*~0.2-magnitude logits). z matmuls carry a ones-column (M=65)
so denominators fall out of PSUM; diagonal blocks use column-trimmed score/z
matmuls, constant-triangle masks and small dead-column memsets. 1/sqrt(dk) is
folded into w_q host-side. Output is f16 (summed/transposed on host).
"""

import numpy as np
import ml_dtypes

import concourse.bacc as bacc
import concourse.mybir as mybir
import concourse.tile as tile
from concourse.bass_utils import run_bass_kernel_spmd

F32 = mybir.dt.float32
F16 = mybir.dt.float16
F8 = mybir.dt.float8e4
AF = mybir.ActivationFunctionType
DR = mybir.MatmulPerfMode.DoubleRow
ALU = mybir.AluOpType

L = 2048
D = 512
HD = 256
DK = 64
NH = 4
P = 128
IB = 512
NIB = L // IB          # 4 query blocks
NKT = D // P           # 4 model-dim tiles
ZW = DK + 1            # z matmul M (64 v dims + ones row -> denominator)

TRI_GPS_EVERY = 1      # every k-th triangle mask runs on gpsimd


def _build():
    nc = bacc.Bacc("TRN2", target_bir_lowering=False, debug=False,
                   enable_asserts=False)

    xT = nc.dram_tensor("xT", [D, L], F16, kind="ExternalInput")
    wq = nc.dram_tensor("wq", [D, HD], F16, kind="ExternalInput")
    wk = nc.dram_tensor("wk", [D, HD], F16, kind="ExternalInput")
    wv = nc.dram_tensor("wv", [D, HD], F16, kind="ExternalInput")
    wo = nc.dram_tensor("wo", [HD, D], F16, kind="ExternalInput")
    bq = nc.dram_tensor("bq", [HD], F32, kind="ExternalInput")
    bk = nc.dram_tensor("bk", [HD], F32, kind="ExternalInput")
    outT = nc.dram_tensor("outT", [D, L], F16, kind="ExternalOutput")
    scr2 = nc.dram_tensor("scr2", [NIB * NH, IB], F32, kind="Internal")

    counters = {"tri": 0}

    with tile.TileContext(nc) as tc:
        with (
            tc.tile_pool(name="w", bufs=1) as pw,
            tc.tile_pool(name="at", bufs=6) as pats,
            tc.tile_pool(name="zc", bufs=2) as pzc,
            tc.tile_pool(name="nm", bufs=2) as pnm,
            tc.tile_pool(name="bc", bufs=2) as pbct,
            tc.tile_pool(name="o", bufs=2) as posb,
            tc.tile_pool(name="ps", bufs=2, space="PSUM") as pps,
            tc.tile_pool(name="pz", bufs=2, space="PSUM") as ppz,
            tc.tile_pool(name="pp", bufs=2, space="PSUM") as ppp,
        ):
            # ---- loads ----
            wq_sb = pw.tile([P, NKT, HD], F16, tag="wq")
            wk_sb = pw.tile([P, NKT, HD], F16, tag="wk")
            wv_sb = pw.tile([P, NKT, HD], F16, tag="wv")
            wo_sb = pw.tile([P, HD // P, D], F16, tag="wo")
            bq_sb = pw.tile([P, HD // P], F32, tag="bq")
            bk_sb = pw.tile([P, HD // P], F32, tag="bk")
            nc.sync.dma_start(bq_sb[:], bq.ap().rearrange("(t p) -> p t", p=P))
            nc.sync.dma_start(bk_sb[:], bk.ap().rearrange("(t p) -> p t", p=P))
            # interleave per-k-tile weight/x chunks so qkproj kt=t can start
            # as soon as its slice lands instead of after whole-tensor loads
            xt = pw.tile([P, NKT, L], F16, tag="xt")
            for t in range(NKT):
                nc.sync.dma_start(wq_sb[:, t, :], wq.ap()[t * P:(t + 1) * P, :])
                nc.sync.dma_start(wk_sb[:, t, :], wk.ap()[t * P:(t + 1) * P, :])
                nc.sync.dma_start(xt[:, t, 0:IB],
                                  xT.ap()[t * P:(t + 1) * P, 0:IB])
            for ibx in range(1, NIB):
                for t in range(NKT):
                    nc.sync.dma_start(
                        xt[:, t, ibx * IB:(ibx + 1) * IB],
                        xT.ap()[t * P:(t + 1) * P, ibx * IB:(ibx + 1) * IB])
            for t in range(NKT):
                nc.sync.dma_start(wv_sb[:, t, :], wv.ap()[t * P:(t + 1) * P, :])
            nc.sync.dma_start(wo_sb[:], wo.ap().rearrange("(t p) n -> p t n", p=P))

            # ---- consts: triangle mask, fp8 q/k mega tile with zero plane ----
            # mega planes: 0,1 = q head-pairs (01, 23), 2,3 = k head-pairs,
            # 4 = zeros (the shared second DoubleRow k-tile). Head h lives on
            # partitions (h%2)*64 .. +64 of plane h//2 (q) / 2+h//2 (k).
            mega = pw.tile([P, 5, L], F8, tag="mega")
            nc.gpsimd.memset(mega[:, 4, :], 0.0)
            tri16 = pw.tile([P, P], F16, tag="tri16")
            nc.gpsimd.memset(tri16[:], 1.0)
            # keep element iff col - row >= 0
            nc.gpsimd.affine_select(tri16[:], tri16[:], pattern=[[1, P]],
                                    compare_op=ALU.is_ge, fill=0.0,
                                    base=0, channel_multiplier=-1)

            def q_ap(h, cols):
                b0 = (h % 2) * DK
                pl = h // 2
                return mega[b0:b0 + DK, pl:5:(4 - pl), cols]

            def k_ap(h, cols):
                b0 = (h % 2) * DK
                pl = 2 + h // 2
                return mega[b0:b0 + DK, pl:5:(4 - pl), cols]

            def qkproj(hp, is_k, ib):
                plane = (2 + hp) if is_k else hp
                w_sb = wk_sb if is_k else wq_sb
                b_sb = bk_sb if is_k else bq_sb
                pq = ppp.tile([P, IB], F32, tag="pp", name="pp")
                for t in range(NKT):
                    nc.tensor.matmul(
                        pq[:],
                        lhsT=w_sb[:, t, hp * P:(hp + 1) * P],
                        rhs=xt[:, t, ib * IB:(ib + 1) * IB],
                        start=(t == 0), stop=(t == NKT - 1))
                nc.vector.tensor_scalar_add(
                    mega[:, plane, ib * IB:(ib + 1) * IB], pq[:],
                    b_sb[:, hp:hp + 1])

            vts = {}

            def vproj(jp):
                vt = pw.tile([P, 2, NH, ZW], F16, tag=f"v{jp}", name=f"v{jp}")
                vts[jp] = vt
                for s in range(2):
                    jt = 2 * jp + s
                    pv = ppp.tile([P, HD], F32, tag="pp", name="pp")
                    for t in range(NKT):
                        nc.tensor.matmul(
                            pv[:],
                            lhsT=xt[:, t, jt * P:(jt + 1) * P],
                            rhs=wv_sb[:, t, :],
                            start=(t == 0), stop=(t == NKT - 1))
                    nc.vector.tensor_copy(
                        vt[:, s, :, 0:DK],
                        pv[:].rearrange("p (h e) -> p h e", h=NH))
                nc.gpsimd.memset(vt[:, :, :, DK:ZW], 1.0)

            zcs = {}

            def attn(ib, h):
                nj = 4 * (ib + 1)
                pzt = ppz.tile([ZW, IB], F32, tag="pz", name="pz")
                pending = []

                def emit_z(jt, att, t, c0):
                    nc.tensor.matmul(
                        pzt[:, c0:IB],
                        lhsT=vts[jt // 2][:, jt % 2, h, :],
                        rhs=att[:, t, c0:IB],
                        start=(jt == 0), stop=(jt == nj - 1),
                        skip_group_check=True)

                for g in range(nj // 2):
                    pst = pps.tile([P, 2, IB], F32, tag="ps", name="ps")
                    att = pats.tile([P, 2, IB], F16, tag="at", name="at")
                    for t in range(2):
                        jt = 2 * g + t
                        d = jt - 4 * ib
                        c0 = 128 * d if d > 0 else 0
                        nc.tensor.matmul(
                            pst[:, t, c0:IB],
                            lhsT=k_ap(h, slice(jt * P, (jt + 1) * P)),
                            rhs=q_ap(h, slice(ib * IB + c0, (ib + 1) * IB)),
                            start=True, stop=True, perf_mode=DR)
                    # one exp per pair over the pair's live-col union;
                    # remaining garbage cols are never read by the
                    # column-trimmed z matmuls below
                    d0 = 2 * g - 4 * ib
                    cE = 128 * d0 if d0 > 0 else 0
                    nc.scalar.activation(att[:, :, cE:IB], pst[:, :, cE:IB],
                                         AF.Exp)
                    for t in range(2):
                        jt = 2 * g + t
                        d = jt - 4 * ib
                        c0 = 128 * d if d > 0 else 0
                        if d >= 0:
                            teng = (nc.gpsimd if TRI_GPS_EVERY and
                                    counters["tri"] % TRI_GPS_EVERY == 1
                                    else nc.vector)
                            counters["tri"] += 1
                            teng.tensor_mul(att[:, t, c0:c0 + P],
                                            att[:, t, c0:c0 + P], tri16[:])
                        if len(pending) >= 3:
                            emit_z(*pending.pop(0))
                        pending.append((jt, att, t, c0))
                for pg in pending:
                    emit_z(*pg)

                # ---- z+den out of PSUM; 1/den; bct via dram replicate ----
                idx = ib * NH + h
                dsb = pnm.tile([P, IB], F32, tag="dsb", name="dsb")
                nc.vector.tensor_copy(dsb[0:ZW, :], pzt[0:ZW, :])
                rec = pnm.tile([P, IB], F32, tag="rec", name="rec")
                rc2 = pnm.tile([P, IB], F32, tag="rc2", name="rc2")
                nc.sync.dma_start(rec[0:1, :], dsb[DK:ZW, :])
                nc.vector.reciprocal_approx_fast(rc2[0:1, :], rec[0:1, :])
                nc.sync.dma_start(scr2.ap()[idx:idx + 1, :], rc2[0:1, :])
                bct = pbct.tile([DK, IB], F32, tag="bct", name="bct")
                nc.gpsimd.dma_start(
                    bct[:], scr2.ap()[idx:idx + 1, :].broadcast_to([DK, IB]))

                zc = zcs[ib][h // 2]
                if h % 2 == 0:
                    nc.vector.tensor_mul(zc[0:DK, :], dsb[0:DK, :], bct[:])
                else:
                    zn = pnm.tile([DK, IB], F16, tag="zn", name="zn")
                    nc.vector.tensor_mul(zn[:], dsb[0:DK, :], bct[:])
                    nc.gpsimd.dma_start(zc[DK:P, :], zn[:])

            def new_zc(ib):
                zcs[ib] = [pzc.tile([P, IB], F16, tag=f"zc{i}", name=f"zc{i}")
                           for i in range(HD // P)]

            def outproj(ib):
                zc = zcs[ib]
                for mt in range(D // P):
                    po = ppp.tile([P, IB], F32, tag="pp", name="pp")
                    for k2 in range(HD // P):
                        nc.tensor.matmul(
                            po[:],
                            lhsT=wo_sb[:, k2, mt * P:(mt + 1) * P],
                            rhs=zc[k2][:],
                            start=(k2 == 0), stop=(k2 == HD // P - 1))
                    osb = posb.tile([P, IB], F16, tag="o", name="o")
                    if mt % 2 == 0:
                        nc.scalar.copy(osb[:], po[:])
                    else:
                        nc.vector.tensor_copy(osb[:], po[:])
                    nc.gpsimd.dma_start(
                        outT.ap()[mt * P:(mt + 1) * P,
                                  ib * IB:(ib + 1) * IB], osb[:])

            # ---- emission schedule ----
            qkproj(0, True, 0)
            qkproj(0, False, 0)
            vproj(0)
            vproj(1)
            new_zc(0)
            attn(0, 1)
            qkproj(1, True, 0)
            qkproj(1, False, 0)
            attn(0, 0)
            attn(0, 3)
            for jp in range(2, 8):
                vproj(jp)
            attn(0, 2)
            for ib in range(1, NIB):
                for hp in range(2):
                    qkproj(hp, True, ib)
                    qkproj(hp, False, ib)
            outproj(0)
            for ib in range(1, NIB):
                new_zc(ib)
                for h in (1, 0, 3, 2):
                    attn(ib, h)
                outproj(ib)

    nc.compile()
    return nc


_NC = None


def _get_nc():
    global _NC
    if _NC is None:
        _NC = _build()
    return _NC


def _in_maps(x, w_q, b_q, w_k, b_k, w_v, b_v, w_o, b_o):
    scale = 1.0 / np.sqrt(DK)
    maps = []
    for b in range(4):
        xTb = np.ascontiguousarray(x[b].T).astype(np.float16)
        for hg in range(2):
            sl = slice(hg * HD, (hg + 1) * HD)
            maps.append({
                "xT": xTb,
                "wq": np.ascontiguousarray((w_q[sl] * scale).T).astype(np.float16),
                "wk": np.ascontiguousarray(w_k[sl].T).astype(np.float16),
                "wv": np.ascontiguousarray(w_v[sl].T).astype(np.float16),
                "wo": np.ascontiguousarray(w_o[:, sl].T).astype(np.float16),
                "bq": np.ascontiguousarray(b_q[sl] * scale).astype(np.float32),
                "bk": np.ascontiguousarray(b_k[sl]).astype(np.float32),
            })
    return maps


def _combine(results, w_o, b_v, b_o):
    corr = (b_o + w_o @ b_v).astype(np.float32)
    out = np.empty((4, L, D), dtype=np.float32)
    for b in range(4):
        acc = (results[2 * b]["outT"].astype(np.float32)
               + results[2 * b + 1]["outT"].astype(np.float32))
        out[b] = acc.T + corr
    return out


def kernel(x, w_q, b_q, w_k, b_k, w_v, b_v, w_o, b_o):
    nc = _get_nc()
    maps = _in_maps(x, w_q, b_q, w_k, b_k, w_v, b_v, w_o, b_o)
    res = run_bass_kernel_spmd(nc, maps, core_ids=list(range(8)))
    return _combine(res.results, w_o, b_v, b_o)


def bench(x, w_q, b_q, w_k, b_k, w_v, b_v, w_o, b_o):
    """Run with NTFF tracing; returns (output, exec_time_ns)."""
    nc = _get_nc()
    maps = _in_maps(x, w_q, b_q, w_k, b_k, w_v, b_v, w_o, b_o)
    res = run_bass_kernel_spmd(nc, maps, core_ids=list(range(8)), trace=True)
    return _combine(res.results, w_o, b_v, b_o), res.exec_time_ns
